# revision 7
# baseline (speedup 1.0000x reference)
"""Trainium2 Bass kernel for CausalSelectiveSelfAttentionWithMemoryPenalty.

Model (B=2, T=2048, C=768, H=12, d=64):
  qkv = x @ W_attn + b_attn ; q,k,v per head
  att = causal(q k^T / 8) ; S = relu(att[:,0]) with col0/diag zeroed
  FF[t,s] = sum_{t'<t} S[t',s]  (per batch, head independent)
  y = proj(softmax(att - FF) v) ; M[0,b,t,s] = t - sum_s' clip(FF[s,s'],0,1)

Sharding: 8 cores = 2 batches x 4 head-groups (3 heads each). Each core
redundantly computes the head-0 S/FF penalty for its batch (float32r
precision), runs its 3 heads' attention in a transposed [key,query]
layout (so the cumsum is a DVE prefix-scan along the free axis and the
softmax denominator falls out of a ones-augmented AV matmul), and emits
a partial y (summed on host over the 4 cores of the batch) plus a
4*128-row slice of M.

Layout notes: matmul requires lhsT/rhs at the same base partition, so
heads 0,1 share [128,T] q/k tiles (h0 in rows 0:64, h1 in rows 64:128)
and head 2 and the S-path q0/k0 live in separate 64-row tiles.
No max-subtraction in the softmax: logits are bounded above (~5) since
FF >= 0, so exp() cannot overflow and the diagonal keeps l > 0.
"""

import sys
import threading

sys.path.insert(0, "/opt/trn_rl_repo")

import numpy as np
import ml_dtypes  # noqa: F401  (np bfloat16 support)

import concourse.bass as bass
import concourse.mybir as mybir
import concourse.tile as tile
from concourse import bacc
from concourse.bass_utils import run_bass_kernel_spmd

F32 = mybir.dt.float32
F32R = mybir.dt.float32r
BF16 = mybir.dt.bfloat16
AF = mybir.ActivationFunctionType
ALU = mybir.AluOpType
ts, ds = bass.ts, bass.ds

B, T, C, H, D = 2, 2048, 768, 12, 64
HPC = 3            # heads per core
NSC = T // 128     # 16 s-chunks
NTS = T // 512     # 4 t-spans
KC = 7             # K chunks over C (6x128) + bias row chunk
N_CORES = 8

_cache = {}
_lock = threading.Lock()


def _build():
    nc = bacc.Bacc(None, target_bir_lowering=False)
    with tile.TileContext(nc) as tc:
        with tc.tile_pool(name="dram", bufs=1, space="DRAM") as dram:
            def din(name, shape, dt):
                return dram.tile(shape, dt, kind="ExternalInput", name=name, uniquify=False)

            xTa_d = din("xTa", [128, KC, T], F32)
            xb_d = din("xb", [128, KC, T], BF16)
            wS_d = din("wS", [128, KC, 128], F32)
            wQK_d = din("wQK", [128, KC, 384], BF16)
            wV_d = din("wV", [128, KC, 192], BF16)
            wP_d = din("wP", [128, 3, 768], BF16)
            mS_d = din("mS", [128, 5, 512], F32)
            mE_d = din("mE", [128, 4, 512], BF16)
            tio_d = din("tio", [128, 4], F32)
            yp_d = dram.tile([T, C], F32, kind="ExternalOutput", name="yp", uniquify=False)
            Ms_d = dram.tile([512, T], F32, kind="ExternalOutput", name="Ms", uniquify=False)

            with tc.tile_pool(name="const", bufs=1) as cst:
                wS = cst.tile([128, KC, 128], F32)
                wQK = cst.tile([128, KC, 384], BF16)
                wV = cst.tile([128, KC, 192], BF16)
                wP = cst.tile([128, 3, 768], BF16)
                mS = cst.tile([128, 5, 512], F32)
                mE = cst.tile([128, 4, 512], BF16)
                tio = cst.tile([128, 4], F32)
                for t_, d_ in ((wS, wS_d), (wQK, wQK_d), (wV, wV_d),
                               (wP, wP_d), (mS, mS_d), (mE, mE_d), (tio, tio_d)):
                    nc.sync.dma_start(t_[:], d_[:])
                ones_f = cst.tile([128, 128], F32)
                nc.vector.memset(ones_f[:], 1.0)
                ones_all = cst.tile([128, 128], F32R)
                nc.vector.tensor_copy(ones_all[:], ones_f[:])
                ones_bc = cst.tile([128, 1], BF16)
                nc.vector.memset(ones_bc[:], 1.0)
                wSr = cst.tile([128, KC, 128], F32R)
                nc.vector.tensor_copy(wSr[:], wS[:])

                # live across phases
                q0t = cst.tile([64, T], F32R)
                k0t = cst.tile([64, T], F32R)
                q01 = cst.tile([128, T], BF16)   # rows 0:64 qT_h0, 64:128 qT_h1
                k01 = cst.tile([128, T], BF16)
                q2t = cst.tile([64, T], BF16)
                k2t = cst.tile([64, T], BF16)
                vaug = cst.tile([128, NSC * HPC, 65], BF16)
                ffrow = cst.tile([1, T], F32R)
                yTn = [cst.tile([64, T], BF16, name=f"yTn{h}") for h in range(HPC)]
                lrow_t = cst.tile([65, 512], F32R)

                # ---- P0: q0/k0 transposed projection in f32r ----
                with tc.tile_pool(name="p0x", bufs=1) as p0x, \
                     tc.tile_pool(name="p0s", bufs=2) as p0s, \
                     tc.tile_pool(name="p0p", bufs=2, space="PSUM") as p0p:
                    xr = p0x.tile([128, KC, T], F32R)
                    xb = p0x.tile([128, KC, T], BF16)
                    nc.sync.dma_start(xb[:], xb_d[:])
                    for c in range(KC):
                        xa_c = p0s.tile([128, T], F32, tag="xa")
                        nc.sync.dma_start(xa_c[:], xTa_d[:, c, :])
                        nc.vector.tensor_copy(xr[:, c, :], xa_c[:])
                    for p in range(NTS):
                        for dst, lo in ((q0t, 0), (k0t, 64)):
                            ps = p0p.tile([64, 512], F32, tag="ps0")
                            for c in range(KC):
                                nc.tensor.matmul(ps[:], wSr[:, c, ds(lo, 64)],
                                                 xr[:, c, ts(p, 512)],
                                                 start=(c == 0), stop=(c == KC - 1))
                            nc.any.tensor_copy(dst[:, ts(p, 512)], ps[:])

                    # ---- P1: heads qkT (bf16) + v natural -> vaug ----
                    for p in range(NTS):
                        for dst, lo, m in ((q01, 0, 128), (k01, 128, 128),
                                           (q2t, 256, 64), (k2t, 320, 64)):
                            ps = p0p.tile([128, 512], F32, tag="ps1")
                            for c in range(KC):
                                nc.tensor.matmul(ps[0:m, :], wQK[:, c, ds(lo, m)],
                                                 xb[:, c, ts(p, 512)],
                                                 start=(c == 0), stop=(c == KC - 1))
                            nc.any.tensor_copy(dst[:, ts(p, 512)], ps[0:m, :])
                    for sc in range(NSC):
                        psv = p0p.tile([128, 192], F32, tag="psv")
                        for c in range(KC):
                            nc.tensor.matmul(psv[:], xb[:, c, ts(sc, 128)], wV[:, c, :],
                                             start=(c == 0), stop=(c == KC - 1))
                        for h in range(HPC):
                            nc.any.tensor_copy(vaug[:, HPC * sc + h, 0:64], psv[:, ts(h, 64)])
                            nc.vector.memset(vaug[:, HPC * sc + h, 64:65], 1.0)

                # ---- P2: S scores, FF prefix-scan, E=exp(-FF), u=min(FF,1), FFsum ----
                pe_pool = tc.alloc_tile_pool(name="pe", bufs=1)
                E = pe_pool.tile([128, NSC, T], BF16)
                with tc.tile_pool(name="p2s", bufs=2) as p2s, \
                     tc.tile_pool(name="p2p", bufs=3, space="PSUM") as p2p, \
                     tc.tile_pool(name="p2f", bufs=1, space="PSUM") as p2f:
                    ffps = [p2f.tile([1, 512], F32, tag=f"ff{p}", name=f"ffps{p}")
                            for p in range(NTS)]
                    for sc in range(NSC):
                        p0_ = sc // 4
                        t0 = 512 * p0_
                        W = T - t0
                        st = p2s.tile([128, T], F32, tag="st")
                        ff = p2s.tile([128, T], F32, tag="ff")
                        u = p2s.tile([128, T], BF16, tag="u")
                        for i in range(W // 512):
                            ps = p2p.tile([128, 512], F32, tag="stp")
                            nc.tensor.matmul(ps[:], k0t[:, ts(sc, 128)],
                                             q0t[:, ds(t0 + 512 * i, 512)],
                                             start=True, stop=True)
                            mv = (sc % 4) if i == 0 else 4
                            nc.vector.scalar_tensor_tensor(
                                st[:, ts(i, 512)], ps[:], 0.0, mS[:, mv, :],
                                ALU.max, ALU.mult)
                        if sc == 0:
                            nc.vector.memset(st[0:1, 0:W], 0.0)
                        nc.vector.tensor_tensor_scan(
                            ff[:, ds(1, W - 1)], st[:, ds(0, W - 1)],
                            st[:, ds(0, W - 1)], 0.0, ALU.add, ALU.bypass)
                        nc.vector.memset(ff[:, 0:1], 0.0)
                        nc.scalar.activation(E[:, sc, ds(0, W)], ff[:, ds(0, W)],
                                             AF.Exp, scale=-1.0)
                        nc.vector.tensor_mul(E[:, sc, 0:512], E[:, sc, 0:512],
                                             mE[:, sc % 4, :])
                        nc.vector.tensor_scalar_min(u[:, ds(0, W)], ff[:, ds(0, W)], 1.0)
                        for i in range(W // 512):
                            p = p0_ + i
                            nc.tensor.matmul(ffps[p][:], ones_bc[:], u[:, ts(i, 512)],
                                             start=(sc == 0), stop=(sc == 4 * p + 3))
                    for p in range(NTS):
                        nc.any.tensor_copy(ffrow[:, ts(p, 512)], ffps[p][:])

                # ---- P3: attention per t-span ----
                qk = [(q01[0:64, :], k01[0:64, :]), (q01[64:128, :], k01[64:128, :]),
                      (q2t[:, :], k2t[:, :])]
                with tc.tile_pool(name="p3s", bufs=3) as p3s, \
                     tc.tile_pool(name="p3r", bufs=2) as p3r, \
                     tc.tile_pool(name="p3a", bufs=3, space="PSUM") as p3a, \
                     tc.tile_pool(name="p3y", bufs=1, space="PSUM") as p3y, \
                     tc.tile_pool(name="p3l", bufs=2, space="PSUM") as p3l:
                    for p in range(NTS):
                        yps = [p3y.tile([65, 512], F32, tag=f"y{h}", name=f"yps{h}")
                               for h in range(HPC)]
                        last_sc = 4 * p + 3
                        for sc in range(last_sc + 1):
                            ecol = 512 * p - 512 * (sc // 4)
                            for h in range(HPC):
                                qt_, kt_ = qk[h]
                                aps = p3a.tile([128, 512], F32, tag="att")
                                nc.tensor.matmul(aps[:], kt_[:, ts(sc, 128)],
                                                 qt_[:, ts(p, 512)],
                                                 start=True, stop=True)
                                pt = p3s.tile([128, 512], BF16, tag="pt")
                                nc.scalar.activation(pt[:], aps[:], AF.Exp, scale=0.125)
                                nc.vector.tensor_mul(pt[:], pt[:], E[:, sc, ds(ecol, 512)])
                                nc.tensor.matmul(yps[h][:], vaug[:, HPC * sc + h, :], pt[:],
                                                 start=(sc == 0), stop=(sc == last_sc))
                        for h in range(HPC):
                            nc.any.tensor_copy(lrow_t[64:65, :], yps[h][64:65, :])
                            lb = p3l.tile([64, 512], F32, tag="lb")
                            nc.tensor.matmul(lb[:], ones_all[64:65, 0:64],
                                             lrow_t[64:65, :], start=True, stop=True)
                            rb = p3r.tile([64, 512], F32, tag="rb")
                            nc.vector.reciprocal(rb[:], lb[:])
                            nc.vector.tensor_mul(yTn[h][:, ts(p, 512)],
                                                 yps[h][0:64, :], rb[:])
                pe_pool.release()

                # ---- P4: projection partials ----
                with tc.tile_pool(name="p4s", bufs=3) as p4s, \
                     tc.tile_pool(name="p4p", bufs=2, space="PSUM") as p4p:
                    for tk in range(NSC):
                        for hf in range(2):
                            pp = p4p.tile([128, 384], F32, tag="pp")
                            for h in range(HPC):
                                nc.tensor.matmul(pp[:], yTn[h][:, ts(tk, 128)],
                                                 wP[0:64, h, ts(hf, 384)],
                                                 start=(h == 0), stop=(h == HPC - 1))
                            yo = p4s.tile([128, 384], F32, tag="yo")
                            nc.any.tensor_copy(yo[:], pp[:])
                            nc.sync.dma_start(yp_d[ts(tk, 128), ts(hf, 384)], yo[:])

                    # ---- P5: M slice ----
                    fb_s = p4s.tile([128, T], F32, tag="fb")
                    for i in range(NTS):
                        fb = p4p.tile([128, 512], F32, tag="fbp")
                        nc.tensor.matmul(fb[:], ones_all[0:1, :], ffrow[:, ts(i, 512)],
                                         start=True, stop=True)
                        nc.any.tensor_copy(fb_s[:, ts(i, 512)], fb[:])
                    for mi in range(4):
                        mt = p4s.tile([128, T], F32, tag="mt")
                        nc.vector.tensor_scalar(mt[:], fb_s[:], -1.0, tio[:, mi:mi + 1],
                                                ALU.mult, ALU.add)
                        nc.sync.dma_start(Ms_d[ts(mi, 128), :], mt[:])
    nc.compile()
    return nc


def _prep_inputs(x, W_attn, b_attn, W_proj, b_proj):
    """Per-core input maps. Core i: batch i//4, heads 3*(i%4)+[0..2]."""
    bf = ml_dtypes.bfloat16
    mS = np.zeros((128, 5, 512), np.float32)
    pp, ff = np.arange(128)[:, None], np.arange(512)[None, :]
    for k in range(4):
        mS[:, k, :] = ((pp + 128 * k) < ff) * 0.125
    mS[:, 4, :] = 0.125
    mE = np.zeros((128, 4, 512), np.float32)
    for k in range(4):
        mE[:, k, :] = ((pp + 128 * k) <= ff) * 1.0
    mE = mE.astype(bf)

    def chunked(a):  # [896, N...] -> [128, 7, N...]
        return np.ascontiguousarray(
            a.reshape(KC, 128, *a.shape[1:]).transpose(1, 0, *range(2, a.ndim + 1)))

    in_maps = []
    for core in range(N_CORES):
        b, j = core // 4, core % 4
        heads = [HPC * j + h for h in range(HPC)]
        xT = np.zeros((KC * 128, T), np.float32)
        xT[:C] = x[b].T
        xT[C] = 1.0
        wS = np.zeros((KC * 128, 128), np.float32)
        wS[:C, 0:64] = W_attn[:, 0:64]
        wS[:C, 64:128] = W_attn[:, C:C + 64]
        wS[C, 0:64] = b_attn[0:64]
        wS[C, 64:128] = b_attn[C:C + 64]
        # wQK cols: [q_h0|q_h1 | k_h0|k_h1 | q_h2 | k_h2]
        wQK = np.zeros((KC * 128, 384), np.float32)
        wV = np.zeros((KC * 128, 192), np.float32)
        for i, h in enumerate(heads):
            qo = 64 * i if i < 2 else 256
            ko = 128 + 64 * i if i < 2 else 320
            wQK[:C, qo:qo + 64] = W_attn[:, 64 * h:64 * h + 64]
            wQK[C, qo:qo + 64] = b_attn[64 * h:64 * h + 64]
            wQK[:C, ko:ko + 64] = W_attn[:, C + 64 * h:C + 64 * h + 64]
            wQK[C, ko:ko + 64] = b_attn[C + 64 * h:C + 64 * h + 64]
            wV[:C, 64 * i:64 * i + 64] = W_attn[:, 2 * C + 64 * h:2 * C + 64 * h + 64]
            wV[C, 64 * i:64 * i + 64] = b_attn[2 * C + 64 * h:2 * C + 64 * h + 64]
        # wP: [128, 3, 768], rows 0:64 of chunk h = W_proj rows of head h
        wPm = np.zeros((128, 3, 768), np.float32)
        for i, h in enumerate(heads):
            wPm[0:64, i, :] = W_proj[64 * h:64 * h + 64]
        tio = np.zeros((128, 4), np.float32)
        for mi in range(4):
            tio[:, mi] = 512 * j + 128 * mi + np.arange(128)
        in_maps.append({
            "xTa": chunked(xT),
            "xb": chunked(xT.astype(bf)),
            "wS": chunked(wS),
            "wQK": chunked(wQK.astype(bf)),
            "wV": chunked(wV.astype(bf)),
            "wP": wPm.astype(bf),
            "mS": mS,
            "mE": mE,
            "tio": tio,
        })
    return in_maps


def _get_nc():
    with _lock:
        if "nc" not in _cache:
            _cache["nc"] = _build()
    return _cache["nc"]


def kernel(x, W_attn, b_attn, W_proj, b_proj, _trace=False, _result_box=None):
    x = np.asarray(x, np.float32)
    W_attn = np.asarray(W_attn, np.float32)
    b_attn = np.asarray(b_attn, np.float32)
    W_proj = np.asarray(W_proj, np.float32)
    b_proj = np.asarray(b_proj, np.float32)
    nc = _get_nc()
    in_maps = _prep_inputs(x, W_attn, b_attn, W_proj, b_proj)
    res = run_bass_kernel_spmd(nc, in_maps, list(range(N_CORES)), trace=_trace)
    if _result_box is not None:
        _result_box.append(res)
    y = np.zeros((B, T, C), np.float32)
    M = np.zeros((1, B, T, T), np.float32)
    for core in range(N_CORES):
        b, j = core // 4, core % 4
        y[b] += res.results[core]["yp"]
        M[0, b, 512 * j:512 * (j + 1), :] = res.results[core]["Ms"]
    y += b_proj[None, None, :]
    return y, M
